# revision 23
# baseline (speedup 1.0000x reference)
"""Bidirectional batch-GRU over ragged graph sequences on 8 Trainium2 cores.

Sharding: core = dir*4 + block. Cores 0-3 run the forward GRU on graph
blocks of 128; cores 4-7 run the backward GRU on the same blocks with
time-reversed inputs. All raggedness is carried by host-prepared data
(padded transposed inputs pre-activated with relu(h+bias), per-step
validity masks, segment-max initial state), so one SPMD program serves
all cores.

All matmul operands are bf16 (1 cyc/row at any moving width; rel-err
budget validated at ~6e-3 vs the 2e-2 gate). Per step, the PE runs
bias+x-projection of step t+1 inside step t's recurrence window so the
tensor engine never idles and holds its full p-state clock:
  p_a[g, 0:1536] = bias_row + x_t @ W_ih^T (+ h @ W_hh^T for r,z cols)
  p_b[g, 512]    = b_hn_row + h @ W_hn^T
  r,z = sigmoid(p_a[:, :1024]); n = tanh(gxn + r * p_b)
  h' = n + z*(h - n);  acc += h' * mask[:, t];  hT = transpose(h')
The n-gate x-projection is copied out of PSUM early (ACT) so the DVE
chain runs in bf16 fast mode.
"""

import os
import numpy as np

os.environ.setdefault("NEURON_RT_RESET_CORES", "1")

import concourse.bacc as bacc
import concourse.mybir as mybir
import concourse.tile as tile
from concourse import bass_utils

F32 = mybir.dt.float32
BF16 = mybir.dt.bfloat16
AF = mybir.ActivationFunctionType
ALU = mybir.AluOpType


def _install_ntff_shim():
    """Make trace=True usable: this image's antenv lacks axon_hooks, and
    run_bass_kernel_spmd hard-imports it when tracing is requested."""
    try:
        import antenv.axon_hooks  # noqa: F401
        return
    except ImportError:
        pass
    try:
        import sys
        import types
        import antenv
        mod = types.ModuleType("antenv.axon_hooks")
        mod._hook = None
        mod.set_axon_ntff_profile_hook = lambda h: setattr(mod, "_hook", h)
        mod.get_axon_ntff_profile_hook = lambda: mod._hook
        sys.modules["antenv.axon_hooks"] = mod
        antenv.axon_hooks = mod
        from trn_agent_boot.trn_boot import _ntff_profile_via_ctypes
        hook = _ntff_profile_via_ctypes("/opt/axon/libaxon_pjrt.so")
        if hook is not None:
            mod.set_axon_ntff_profile_hook(hook)
    except Exception:
        pass


_install_ntff_shim()

B, T, H = 512, 128, 512
G3 = 3 * H
BPC = 128             # graphs per core
NCORES = 8
MM_MODE = "bf16"

_CACHE = {}
LAST_RESULTS = None


def _build_program():
    nc = bacc.Bacc("TRN2", target_bir_lowering=False, debug=False,
                   num_devices=NCORES)
    xT = nc.dram_tensor("xT", [128, T * 512], BF16, kind="ExternalInput").ap()
    wx = nc.dram_tensor("wx", [512, G3], BF16, kind="ExternalInput").ap()
    wh = nc.dram_tensor("wh", [512, G3], BF16, kind="ExternalInput").ap()
    biasrow = nc.dram_tensor("biasrow", [1, 2048], BF16,
                             kind="ExternalInput").ap()
    onesr = nc.dram_tensor("onesr", [1, 128], BF16, kind="ExternalInput").ap()
    hT0 = nc.dram_tensor("hT0", [128, 512], BF16, kind="ExternalInput").ap()
    h0g = nc.dram_tensor("h0g", [128, 512], BF16, kind="ExternalInput").ap()
    msk = nc.dram_tensor("msk", [128, T], F32, kind="ExternalInput").ap()
    ident = nc.dram_tensor("ident", [128, 128], BF16, kind="ExternalInput").ap()
    out = nc.dram_tensor("out", [128, 512], F32, kind="ExternalOutput").ap()

    with tile.TileContext(nc) as tc:
        with (
            tc.tile_pool(name="const", bufs=1) as cpool,
            tc.tile_pool(name="xin", bufs=4) as xpool,
            tc.tile_pool(name="gates", bufs=2) as gpool,
            tc.tile_pool(name="state", bufs=2) as spool,
            tc.tile_pool(name="accp", bufs=1) as apool,
            tc.tile_pool(name="pa", bufs=2, space="PSUM") as pa_pool,
            tc.tile_pool(name="pb", bufs=1, space="PSUM") as pb_pool,
            tc.tile_pool(name="pt", bufs=1, space="PSUM") as pt_pool,
        ):
            wx_sb, wh_sb = [], []
            for c in range(4):
                t_ = cpool.tile([128, G3], BF16, tag=f"wx{c}", name=f"wxs{c}")
                nc.sync.dma_start(t_[:], wx[c * 128:(c + 1) * 128, :])
                wx_sb.append(t_)
            for c in range(4):
                t_ = cpool.tile([128, G3], BF16, tag=f"wh{c}", name=f"whs{c}")
                nc.sync.dma_start(t_[:], wh[c * 128:(c + 1) * 128, :])
                wh_sb.append(t_)
            bias_sb = cpool.tile([1, 2048], BF16, tag="biasrow")
            nc.sync.dma_start(bias_sb[:], biasrow[:])
            ones_sb = cpool.tile([1, 128], BF16, tag="ones")
            nc.sync.dma_start(ones_sb[:], onesr[:])
            id_sb = cpool.tile([128, 128], BF16, tag="ident")
            nc.sync.dma_start(id_sb[:], ident[:])
            msk_sb = cpool.tile([128, T], F32, tag="msk")
            nc.sync.dma_start(msk_sb[:], msk[:])

            acc = apool.tile([128, 512], F32, tag="acc")
            nc.vector.memset(acc[:], 0.0)

            hT_prev = spool.tile([128, 512], BF16, tag="hT")
            nc.sync.dma_start(hT_prev[:], hT0[:])
            hg_prev = spool.tile([128, 512], BF16, tag="hg")
            nc.sync.dma_start(hg_prev[:], h0g[:])

            x_tiles = {}

            def fetch_x(t):
                x_t = xpool.tile([128, 512], BF16, tag="x")
                nc.sync.dma_start(x_t[:], xT[:, t * 512:(t + 1) * 512])
                x_tiles[t] = x_t

            pa_tiles = {}

            def emit_bias_a(t):
                """Seed step t's p_a psum banks with the gate bias rows."""
                p_a = pa_tiles[t]
                for nb in range(3):
                    cols = slice(nb * 512, (nb + 1) * 512)
                    nc.tensor.matmul(p_a[:, cols], ones_sb[:],
                                     bias_sb[:, cols], start=True, stop=False)

            def emit_bias_b(p_b):
                nc.tensor.matmul(p_b[:], ones_sb[:], bias_sb[:, 1536:2048],
                                 start=True, stop=False)

            def emit_x(t, region):
                """x-projection of step t for one gate region (0=n,1=r,2=z)."""
                p_a = pa_tiles[t]
                x_t = x_tiles[t]
                nb = (2, 0, 1)[region]
                cols = slice(nb * 512, (nb + 1) * 512)
                for c in range(4):
                    gcols = slice(c * 128, (c + 1) * 128)
                    nc.tensor.matmul(p_a[:, cols], x_t[:, gcols],
                                     wx_sb[c][:, cols],
                                     start=False, stop=(nb == 2 and c == 3))

            # preamble: x(0) fully projected before the scan starts
            fetch_x(0)
            fetch_x(1)
            pa_tiles[0] = pa_pool.tile([128, G3], F32, tag="pa", name="pa0")
            pb_tiles = {0: pb_pool.tile([128, 512], F32, tag="pb", name="pb0")}
            emit_bias_a(0)
            emit_bias_b(pb_tiles[0])
            for reg in range(3):
                emit_x(0, reg)

            for t in range(T):
                p_a = pa_tiles[t]
                p_b = pb_tiles[t]
                if t + 2 < T:
                    fetch_x(t + 2)

                # --- recurrent matmuls (wait on hT_prev): r, nh, z ---
                for c in range(4):
                    gcols = slice(c * 128, (c + 1) * 128)
                    nc.tensor.matmul(p_a[:, 0:512],
                                     hT_prev[:, gcols],
                                     wh_sb[c][:, 0:512],
                                     start=False, stop=(c == 3))
                for c in range(4):
                    gcols = slice(c * 128, (c + 1) * 128)
                    nc.tensor.matmul(p_b[:], hT_prev[:, gcols],
                                     wh_sb[c][:, 1024:1536],
                                     start=False, stop=(c == 3))
                for c in range(4):
                    gcols = slice(c * 128, (c + 1) * 128)
                    nc.tensor.matmul(p_a[:, 512:1024],
                                     hT_prev[:, gcols],
                                     wh_sb[c][:, 512:1024],
                                     start=False, stop=(c == 3))

                # --- ACT: sig-r, sig-z, tanh (tanh emitted below) ---
                r_sb = gpool.tile([128, 512], BF16, tag="r")
                nc.scalar.activation(r_sb[:], p_a[:, 0:512], AF.Sigmoid)
                z_sb = gpool.tile([128, 512], BF16, tag="z")
                nc.scalar.activation(z_sb[:], p_a[:, 512:1024], AF.Sigmoid)

                # gxn copy on DVE (input ready since last iteration)
                gxn = gpool.tile([128, 512], BF16, tag="gxn")
                nc.vector.tensor_copy(gxn[:], p_a[:, 1024:1536])

                # --- x-projection of t+1 fills the PE during the chain ---
                if t + 1 < T:
                    pa_tiles[t + 1] = pa_pool.tile([128, G3], F32, tag="pa",
                                                   name=f"pa{t + 1}")
                    pb_tiles[t + 1] = pb_pool.tile([128, 512], F32, tag="pb",
                                                   name=f"pb{t + 1}")
                    emit_bias_a(t + 1)
                    for reg in range(3):
                        emit_x(t + 1, reg)

                t2 = gpool.tile([128, 512], BF16, tag="t2")
                nc.vector.tensor_mul(t2[:], r_sb[:], p_b[:])
                t3 = gpool.tile([128, 512], BF16, tag="t3")
                nc.vector.tensor_add(t3[:], t2[:], gxn[:])
                n_sb = gpool.tile([128, 512], BF16, tag="n")
                nc.scalar.activation(n_sb[:], t3[:], AF.Tanh)
                # h' = z*h + (1-z)*n   (z', q1 computed off the critical path)
                zc = gpool.tile([128, 512], BF16, tag="zc")
                nc.vector.tensor_scalar(zc[:], z_sb[:], -1.0, 1.0,
                                        op0=ALU.mult, op1=ALU.add)
                q1 = gpool.tile([128, 512], BF16, tag="q1")
                nc.vector.tensor_mul(q1[:], z_sb[:], hg_prev[:])
                q2 = gpool.tile([128, 512], BF16, tag="q2")
                nc.vector.tensor_mul(q2[:], zc[:], n_sb[:])
                hg = spool.tile([128, 512], BF16, tag="hg")
                nc.vector.tensor_add(hg[:], q1[:], q2[:])

                if t + 1 < T:
                    # p_b bias for t+1 here: t2(t) has read p_b by now, and
                    # this keeps the PE busy right before the transposes
                    emit_bias_b(pb_tiles[t + 1])
                    p_t = pt_pool.tile([128, 512], BF16, tag="pt")
                    for c in range(4):
                        gcols = slice(c * 128, (c + 1) * 128)
                        nc.tensor.transpose(p_t[:, gcols],
                                            hg[:, gcols], id_sb[:])
                    hT = spool.tile([128, 512], BF16, tag="hT")
                    nc.vector.tensor_copy(hT[:], p_t[:])
                    hT_prev = hT

                # acc += h' * mask[:, t]
                nc.vector.scalar_tensor_tensor(
                    acc[:], hg[:], msk_sb[:, t:t + 1], acc[:],
                    op0=ALU.mult, op1=ALU.add)
                hg_prev = hg

            nc.sync.dma_start(out[:], acc[:])

    nc.compile()
    return nc


def _host_prep(msg, lengths, block, direction, starts, h0_all, bf):
    """Build one core's input map."""
    gs = block * BPC
    lens = lengths[gs:gs + BPC]
    sts = starts[gs:gs + BPC]

    xpad = np.zeros((T, BPC, H), np.float32)
    mask = np.zeros((BPC, T), np.float32)
    node_rows = np.concatenate(
        [np.arange(sts[j], sts[j] + lens[j]) for j in range(BPC)])
    g_idx = np.repeat(np.arange(BPC), lens)
    pos = np.concatenate([np.arange(lens[j]) for j in range(BPC)])
    t_idx = pos if direction == 0 else (T - 1 - pos)
    xpad[t_idx, g_idx] = msg[node_rows]
    if direction == 0:
        mask[g_idx, pos] = 1.0
    else:
        mask[g_idx, T - 1 - pos] = 1.0

    # xT [128, T*512]: row p, col t*512 + c*128 + g  = xpad[t, g, 128c+p]
    xT = np.ascontiguousarray(
        xpad.reshape(T, BPC, 4, 128).transpose(3, 0, 2, 1).reshape(128, T * 512)
    ).astype(bf)

    h0 = h0_all[gs:gs + BPC]                                   # [g, H]
    hT0 = np.ascontiguousarray(
        h0.reshape(BPC, 4, 128).transpose(2, 1, 0).reshape(128, 512)
    ).astype(bf)
    h0g = np.ascontiguousarray(h0).astype(bf)

    return {
        "xT": xT,
        "hT0": hT0,
        "h0g": h0g,
        "msk": mask,
    }


def kernel(**inputs):
    global LAST_RESULTS
    import ml_dtypes
    bf = ml_dtypes.bfloat16

    h = np.asarray(inputs["h"], np.float32)
    lengths = np.asarray(inputs["lengths"]).astype(np.int64)
    bias = np.asarray(inputs["bias"], np.float32)

    starts = np.concatenate([[0], np.cumsum(lengths)[:-1]]).astype(np.int64)
    h0_all = np.maximum.reduceat(h, starts, axis=0)            # segment max
    msg = np.maximum(h + bias, 0.0)

    if "nc" not in _CACHE:
        _CACHE["nc"] = _build_program()
    nc = _CACHE["nc"]

    wkeys = {0: ("w_ih_f", "w_hh_f", "b_ih_f", "b_hh_f"),
             1: ("w_ih_b", "w_hh_b", "b_ih_b", "b_hh_b")}
    shared = {}
    for direction in (0, 1):
        kw, kh, kbi, kbh = wkeys[direction]
        w_ih = np.asarray(inputs[kw], np.float32)
        w_hh = np.asarray(inputs[kh], np.float32)
        b_ih = np.asarray(inputs[kbi], np.float32)
        b_hh = np.asarray(inputs[kbh], np.float32)
        brow = np.concatenate(
            [b_ih + np.concatenate([b_hh[:1024], np.zeros(512, np.float32)]),
             b_hh[1024:]]).astype(np.float32)              # [2048]
        shared[direction] = {
            "wx": np.ascontiguousarray(w_ih.T).astype(bf),
            "wh": np.ascontiguousarray(w_hh.T).astype(bf),
            "biasrow": brow.reshape(1, 2048).astype(bf),
        }
    ones = np.ones((1, 128), np.float32).astype(bf)
    ident = np.eye(128, dtype=np.float32).astype(bf)

    in_maps = []
    for core in range(NCORES):
        direction, block = divmod(core, 4)
        im = _host_prep(msg, lengths, block, direction, starts, h0_all, bf)
        im.update(shared[direction])
        im["onesr"] = ones
        im["ident"] = ident
        in_maps.append(im)

    res = bass_utils.run_bass_kernel_spmd(nc, in_maps,
                                          core_ids=list(range(NCORES)))
    LAST_RESULTS = res

    out = np.zeros((B, 2 * H), np.float32)
    for core in range(NCORES):
        direction, block = divmod(core, 4)
        gs = block * BPC
        acc = np.asarray(res.results[core]["out"], np.float32)  # [g, H]
        cols = slice(0, H) if direction == 0 else slice(H, 2 * H)
        out[gs:gs + BPC, cols] = acc
    out /= lengths[:, None].astype(np.float32)
    return out


# revision 25
# speedup vs baseline: 1.0930x; 1.0930x over previous
"""Bidirectional batch-GRU over ragged graph sequences on 8 Trainium2 cores.

Sharding: core = dir*4 + block. Cores 0-3 run the forward GRU on graph
blocks of 128; cores 4-7 run the backward GRU on the same blocks with
time-reversed inputs. All raggedness is carried by host-prepared data
(padded transposed inputs pre-activated with relu(h+bias), per-step
validity masks, segment-max initial state), so one SPMD program serves
all cores.

All matmul operands are bf16 (1 cyc/row at any moving width; rel-err
budget validated at ~6e-3 vs the 2e-2 gate). Per step, the PE runs
bias+x-projection of step t+1 inside step t's recurrence window so the
tensor engine never idles and holds its full p-state clock:
  p_a[g, 0:1536] = bias_row + x_t @ W_ih^T (+ h @ W_hh^T for r,z cols)
  p_b[g, 512]    = b_hn_row + h @ W_hn^T
  r,z = sigmoid(p_a[:, :1024]); n = tanh(gxn + r * p_b)
  h' = n + z*(h - n);  acc += h' * mask[:, t];  hT = transpose(h')
The n-gate x-projection is copied out of PSUM early (ACT) so the DVE
chain runs in bf16 fast mode.
"""

import os
import numpy as np

os.environ.setdefault("NEURON_RT_RESET_CORES", "1")

import concourse.bacc as bacc
import concourse.mybir as mybir
import concourse.tile as tile
from concourse import bass_utils

F32 = mybir.dt.float32
BF16 = mybir.dt.bfloat16
AF = mybir.ActivationFunctionType
ALU = mybir.AluOpType


def _install_ntff_shim():
    """Make trace=True usable: this image's antenv lacks axon_hooks, and
    run_bass_kernel_spmd hard-imports it when tracing is requested."""
    try:
        import antenv.axon_hooks  # noqa: F401
        return
    except ImportError:
        pass
    try:
        import sys
        import types
        import antenv
        mod = types.ModuleType("antenv.axon_hooks")
        mod._hook = None
        mod.set_axon_ntff_profile_hook = lambda h: setattr(mod, "_hook", h)
        mod.get_axon_ntff_profile_hook = lambda: mod._hook
        sys.modules["antenv.axon_hooks"] = mod
        antenv.axon_hooks = mod
        from trn_agent_boot.trn_boot import _ntff_profile_via_ctypes
        hook = _ntff_profile_via_ctypes("/opt/axon/libaxon_pjrt.so")
        if hook is not None:
            mod.set_axon_ntff_profile_hook(hook)
    except Exception:
        pass


_install_ntff_shim()

B, T, H = 512, 128, 512
G3 = 3 * H
BPC = 128             # graphs per core
NCORES = 8
MM_MODE = "bf16"

_CACHE = {}
LAST_RESULTS = None


def _build_program():
    nc = bacc.Bacc("TRN2", target_bir_lowering=False, debug=False,
                   num_devices=NCORES)
    xT = nc.dram_tensor("xT", [128, T * 512], BF16, kind="ExternalInput").ap()
    wx = nc.dram_tensor("wx", [512, G3], BF16, kind="ExternalInput").ap()
    wh = nc.dram_tensor("wh", [512, G3], BF16, kind="ExternalInput").ap()
    biasrow = nc.dram_tensor("biasrow", [1, 2048], BF16,
                             kind="ExternalInput").ap()
    onesr = nc.dram_tensor("onesr", [1, 128], BF16, kind="ExternalInput").ap()
    hT0 = nc.dram_tensor("hT0", [128, 512], BF16, kind="ExternalInput").ap()
    h0g = nc.dram_tensor("h0g", [128, 512], BF16, kind="ExternalInput").ap()
    msk = nc.dram_tensor("msk", [128, T], F32, kind="ExternalInput").ap()
    ident = nc.dram_tensor("ident", [128, 128], BF16, kind="ExternalInput").ap()
    out = nc.dram_tensor("out", [128, 512], F32, kind="ExternalOutput").ap()

    with tile.TileContext(nc) as tc:
        with (
            tc.tile_pool(name="const", bufs=1) as cpool,
            tc.tile_pool(name="xin", bufs=4) as xpool,
            tc.tile_pool(name="gates", bufs=2) as gpool,
            tc.tile_pool(name="state", bufs=2) as spool,
            tc.tile_pool(name="accp", bufs=1) as apool,
            tc.tile_pool(name="pa", bufs=2, space="PSUM") as pa_pool,
            tc.tile_pool(name="pb", bufs=1, space="PSUM") as pb_pool,
            tc.tile_pool(name="pt", bufs=1, space="PSUM") as pt_pool,
        ):
            wx_sb, wh_sb = [], []
            for c in range(4):
                t_ = cpool.tile([128, G3], BF16, tag=f"wx{c}", name=f"wxs{c}")
                nc.sync.dma_start(t_[:], wx[c * 128:(c + 1) * 128, :])
                wx_sb.append(t_)
            for c in range(4):
                t_ = cpool.tile([128, G3], BF16, tag=f"wh{c}", name=f"whs{c}")
                nc.sync.dma_start(t_[:], wh[c * 128:(c + 1) * 128, :])
                wh_sb.append(t_)
            bias_sb = cpool.tile([1, 2048], BF16, tag="biasrow")
            nc.sync.dma_start(bias_sb[:], biasrow[:])
            ones_sb = cpool.tile([1, 128], BF16, tag="ones")
            nc.sync.dma_start(ones_sb[:], onesr[:])
            id_sb = cpool.tile([128, 128], BF16, tag="ident")
            nc.sync.dma_start(id_sb[:], ident[:])
            msk_sb = cpool.tile([128, T], F32, tag="msk")
            nc.sync.dma_start(msk_sb[:], msk[:])

            acc = apool.tile([128, 512], F32, tag="acc")
            nc.vector.memset(acc[:], 0.0)

            hT_prev = spool.tile([128, 512], BF16, tag="hT")
            nc.sync.dma_start(hT_prev[:], hT0[:])
            hg_prev = spool.tile([128, 512], BF16, tag="hg")
            nc.sync.dma_start(hg_prev[:], h0g[:])

            x_tiles = {}

            def fetch_x(t):
                x_t = xpool.tile([128, 512], BF16, tag="x")
                nc.sync.dma_start(x_t[:], xT[:, t * 512:(t + 1) * 512])
                x_tiles[t] = x_t

            pa_tiles = {}

            def emit_bias_a(t):
                """Seed step t's p_a psum banks with the gate bias rows."""
                p_a = pa_tiles[t]
                for nb in range(3):
                    cols = slice(nb * 512, (nb + 1) * 512)
                    nc.tensor.matmul(p_a[:, cols], ones_sb[:],
                                     bias_sb[:, cols], start=True, stop=False)

            def emit_bias_b(p_b):
                nc.tensor.matmul(p_b[:], ones_sb[:], bias_sb[:, 1536:2048],
                                 start=True, stop=False)

            def emit_x(t, region):
                """x-projection of step t for one gate region (0=n,1=r,2=z)."""
                p_a = pa_tiles[t]
                x_t = x_tiles[t]
                nb = (2, 0, 1)[region]
                cols = slice(nb * 512, (nb + 1) * 512)
                for c in range(4):
                    gcols = slice(c * 128, (c + 1) * 128)
                    nc.tensor.matmul(p_a[:, cols], x_t[:, gcols],
                                     wx_sb[c][:, cols],
                                     start=False, stop=(nb == 2 and c == 3))

            # preamble: x(0) fully projected before the scan starts
            fetch_x(0)
            fetch_x(1)
            pa_tiles[0] = pa_pool.tile([128, G3], F32, tag="pa", name="pa0")
            pb_tiles = {0: pb_pool.tile([128, 512], F32, tag="pb", name="pb0")}
            emit_bias_a(0)
            emit_bias_b(pb_tiles[0])
            for reg in range(3):
                emit_x(0, reg)

            for t in range(T):
                p_a = pa_tiles[t]
                p_b = pb_tiles[t]
                if t + 2 < T:
                    fetch_x(t + 2)

                # --- recurrent matmuls (wait on hT_prev): r, nh, z ---
                for c in range(4):
                    gcols = slice(c * 128, (c + 1) * 128)
                    nc.tensor.matmul(p_a[:, 0:512],
                                     hT_prev[:, gcols],
                                     wh_sb[c][:, 0:512],
                                     start=False, stop=(c == 3))
                for c in range(4):
                    gcols = slice(c * 128, (c + 1) * 128)
                    nc.tensor.matmul(p_b[:], hT_prev[:, gcols],
                                     wh_sb[c][:, 1024:1536],
                                     start=False, stop=(c == 3))
                for c in range(4):
                    gcols = slice(c * 128, (c + 1) * 128)
                    nc.tensor.matmul(p_a[:, 512:1024],
                                     hT_prev[:, gcols],
                                     wh_sb[c][:, 512:1024],
                                     start=False, stop=(c == 3))

                # --- ACT: sig-r, sig-z, tanh (tanh emitted below) ---
                r_sb = gpool.tile([128, 512], BF16, tag="r")
                nc.scalar.activation(r_sb[:], p_a[:, 0:512], AF.Sigmoid)
                z_sb = gpool.tile([128, 512], BF16, tag="z")
                nc.scalar.activation(z_sb[:], p_a[:, 512:1024], AF.Sigmoid)

                # drain p_b to SBUF immediately so its bank is free for the
                # next step's bias (keeps the PE queue from stalling)
                nh_sb = gpool.tile([128, 512], BF16, tag="nh")
                nc.vector.tensor_copy(nh_sb[:], p_b[:])

                # --- x-projection of t+1 fills the PE during the chain ---
                if t + 1 < T:
                    pa_tiles[t + 1] = pa_pool.tile([128, G3], F32, tag="pa",
                                                   name=f"pa{t + 1}")
                    pb_tiles[t + 1] = pb_pool.tile([128, 512], F32, tag="pb",
                                                   name=f"pb{t + 1}")
                    emit_bias_a(t + 1)
                    for reg in range(3):
                        emit_x(t + 1, reg)

                t2 = gpool.tile([128, 512], BF16, tag="t2")
                nc.vector.tensor_mul(t2[:], r_sb[:], nh_sb[:])
                t3 = gpool.tile([128, 512], BF16, tag="t3")
                nc.vector.tensor_add(t3[:], t2[:], p_a[:, 1024:1536])
                n_sb = gpool.tile([128, 512], BF16, tag="n")
                nc.scalar.activation(n_sb[:], t3[:], AF.Tanh)
                # h' = z*h + (1-z)*n   (z', q1 computed off the critical path)
                zc = gpool.tile([128, 512], BF16, tag="zc")
                nc.vector.tensor_scalar(zc[:], z_sb[:], -1.0, 1.0,
                                        op0=ALU.mult, op1=ALU.add)
                q1 = gpool.tile([128, 512], BF16, tag="q1")
                nc.vector.tensor_mul(q1[:], z_sb[:], hg_prev[:])
                q2 = gpool.tile([128, 512], BF16, tag="q2")
                nc.vector.tensor_mul(q2[:], zc[:], n_sb[:])
                hg = spool.tile([128, 512], BF16, tag="hg")
                nc.vector.tensor_add(hg[:], q1[:], q2[:])

                if t + 1 < T:
                    # p_b bias for t+1 here: t2(t) has read p_b by now, and
                    # this keeps the PE busy right before the transposes
                    emit_bias_b(pb_tiles[t + 1])
                    p_t = pt_pool.tile([128, 512], BF16, tag="pt")
                    for c in range(4):
                        gcols = slice(c * 128, (c + 1) * 128)
                        nc.tensor.transpose(p_t[:, gcols],
                                            hg[:, gcols], id_sb[:])
                    hT = spool.tile([128, 512], BF16, tag="hT")
                    nc.vector.tensor_copy(hT[:], p_t[:])
                    hT_prev = hT

                # acc += h' * mask[:, t]
                nc.vector.scalar_tensor_tensor(
                    acc[:], hg[:], msk_sb[:, t:t + 1], acc[:],
                    op0=ALU.mult, op1=ALU.add)
                hg_prev = hg

            nc.sync.dma_start(out[:], acc[:])

    nc.compile()
    return nc


def _host_prep(msg, lengths, block, direction, starts, h0_all, bf):
    """Build one core's input map."""
    gs = block * BPC
    lens = lengths[gs:gs + BPC]
    sts = starts[gs:gs + BPC]

    xpad = np.zeros((T, BPC, H), np.float32)
    mask = np.zeros((BPC, T), np.float32)
    node_rows = np.concatenate(
        [np.arange(sts[j], sts[j] + lens[j]) for j in range(BPC)])
    g_idx = np.repeat(np.arange(BPC), lens)
    pos = np.concatenate([np.arange(lens[j]) for j in range(BPC)])
    t_idx = pos if direction == 0 else (T - 1 - pos)
    xpad[t_idx, g_idx] = msg[node_rows]
    if direction == 0:
        mask[g_idx, pos] = 1.0
    else:
        mask[g_idx, T - 1 - pos] = 1.0

    # xT [128, T*512]: row p, col t*512 + c*128 + g  = xpad[t, g, 128c+p]
    xT = np.ascontiguousarray(
        xpad.reshape(T, BPC, 4, 128).transpose(3, 0, 2, 1).reshape(128, T * 512)
    ).astype(bf)

    h0 = h0_all[gs:gs + BPC]                                   # [g, H]
    hT0 = np.ascontiguousarray(
        h0.reshape(BPC, 4, 128).transpose(2, 1, 0).reshape(128, 512)
    ).astype(bf)
    h0g = np.ascontiguousarray(h0).astype(bf)

    return {
        "xT": xT,
        "hT0": hT0,
        "h0g": h0g,
        "msk": mask,
    }


def kernel(**inputs):
    global LAST_RESULTS
    import ml_dtypes
    bf = ml_dtypes.bfloat16

    h = np.asarray(inputs["h"], np.float32)
    lengths = np.asarray(inputs["lengths"]).astype(np.int64)
    bias = np.asarray(inputs["bias"], np.float32)

    starts = np.concatenate([[0], np.cumsum(lengths)[:-1]]).astype(np.int64)
    h0_all = np.maximum.reduceat(h, starts, axis=0)            # segment max
    msg = np.maximum(h + bias, 0.0)

    if "nc" not in _CACHE:
        _CACHE["nc"] = _build_program()
    nc = _CACHE["nc"]

    wkeys = {0: ("w_ih_f", "w_hh_f", "b_ih_f", "b_hh_f"),
             1: ("w_ih_b", "w_hh_b", "b_ih_b", "b_hh_b")}
    shared = {}
    for direction in (0, 1):
        kw, kh, kbi, kbh = wkeys[direction]
        w_ih = np.asarray(inputs[kw], np.float32)
        w_hh = np.asarray(inputs[kh], np.float32)
        b_ih = np.asarray(inputs[kbi], np.float32)
        b_hh = np.asarray(inputs[kbh], np.float32)
        brow = np.concatenate(
            [b_ih + np.concatenate([b_hh[:1024], np.zeros(512, np.float32)]),
             b_hh[1024:]]).astype(np.float32)              # [2048]
        shared[direction] = {
            "wx": np.ascontiguousarray(w_ih.T).astype(bf),
            "wh": np.ascontiguousarray(w_hh.T).astype(bf),
            "biasrow": brow.reshape(1, 2048).astype(bf),
        }
    ones = np.ones((1, 128), np.float32).astype(bf)
    ident = np.eye(128, dtype=np.float32).astype(bf)

    in_maps = []
    for core in range(NCORES):
        direction, block = divmod(core, 4)
        im = _host_prep(msg, lengths, block, direction, starts, h0_all, bf)
        im.update(shared[direction])
        im["onesr"] = ones
        im["ident"] = ident
        in_maps.append(im)

    res = bass_utils.run_bass_kernel_spmd(nc, in_maps,
                                          core_ids=list(range(NCORES)))
    LAST_RESULTS = res

    out = np.zeros((B, 2 * H), np.float32)
    for core in range(NCORES):
        direction, block = divmod(core, 4)
        gs = block * BPC
        acc = np.asarray(res.results[core]["out"], np.float32)  # [g, H]
        cols = slice(0, H) if direction == 0 else slice(H, 2 * H)
        out[gs:gs + BPC, cols] = acc
    out /= lengths[:, None].astype(np.float32)
    return out


# revision 26
# speedup vs baseline: 1.1162x; 1.0212x over previous
"""Bidirectional batch-GRU over ragged graph sequences on 8 Trainium2 cores.

Sharding: core = dir*4 + block. Cores 0-3 run the forward GRU on graph
blocks of 128; cores 4-7 run the backward GRU on the same blocks with
time-reversed inputs. All raggedness is carried by host-prepared data
(padded transposed inputs pre-activated with relu(h+bias), per-step
validity masks, segment-max initial state), so one SPMD program serves
all cores.

All matmul operands are bf16 (1 cyc/row at any moving width; rel-err
budget validated at ~6e-3 vs the 2e-2 gate). Per step, the PE runs
bias+x-projection of step t+1 inside step t's recurrence window so the
tensor engine never idles and holds its full p-state clock:
  p_a[g, 0:1536] = bias_row + x_t @ W_ih^T (+ h @ W_hh^T for r,z cols)
  p_b[g, 512]    = b_hn_row + h @ W_hn^T
  r,z = sigmoid(p_a[:, :1024]); n = tanh(gxn + r * p_b)
  h' = n + z*(h - n);  acc += h' * mask[:, t];  hT = transpose(h')
The n-gate x-projection is copied out of PSUM early (ACT) so the DVE
chain runs in bf16 fast mode.
"""

import os
import numpy as np

os.environ.setdefault("NEURON_RT_RESET_CORES", "1")

import concourse.bacc as bacc
import concourse.mybir as mybir
import concourse.tile as tile
from concourse import bass_utils

F32 = mybir.dt.float32
BF16 = mybir.dt.bfloat16
AF = mybir.ActivationFunctionType
ALU = mybir.AluOpType


def _install_ntff_shim():
    """Make trace=True usable: this image's antenv lacks axon_hooks, and
    run_bass_kernel_spmd hard-imports it when tracing is requested."""
    try:
        import antenv.axon_hooks  # noqa: F401
        return
    except ImportError:
        pass
    try:
        import sys
        import types
        import antenv
        mod = types.ModuleType("antenv.axon_hooks")
        mod._hook = None
        mod.set_axon_ntff_profile_hook = lambda h: setattr(mod, "_hook", h)
        mod.get_axon_ntff_profile_hook = lambda: mod._hook
        sys.modules["antenv.axon_hooks"] = mod
        antenv.axon_hooks = mod
        from trn_agent_boot.trn_boot import _ntff_profile_via_ctypes
        hook = _ntff_profile_via_ctypes("/opt/axon/libaxon_pjrt.so")
        if hook is not None:
            mod.set_axon_ntff_profile_hook(hook)
    except Exception:
        pass


_install_ntff_shim()

B, T, H = 512, 128, 512
G3 = 3 * H
BPC = 128             # graphs per core
NCORES = 8
MM_MODE = "bf16"

_CACHE = {}
LAST_RESULTS = None


def _build_program():
    nc = bacc.Bacc("TRN2", target_bir_lowering=False, debug=False,
                   num_devices=NCORES)
    xT = nc.dram_tensor("xT", [128, T * 512], BF16, kind="ExternalInput").ap()
    wx = nc.dram_tensor("wx", [512, G3], BF16, kind="ExternalInput").ap()
    wh = nc.dram_tensor("wh", [512, G3], BF16, kind="ExternalInput").ap()
    biasrow = nc.dram_tensor("biasrow", [1, 2048], BF16,
                             kind="ExternalInput").ap()
    onesr = nc.dram_tensor("onesr", [1, 128], BF16, kind="ExternalInput").ap()
    hT0 = nc.dram_tensor("hT0", [128, 512], BF16, kind="ExternalInput").ap()
    h0g = nc.dram_tensor("h0g", [128, 512], BF16, kind="ExternalInput").ap()
    msk = nc.dram_tensor("msk", [128, T], F32, kind="ExternalInput").ap()
    ident = nc.dram_tensor("ident", [128, 128], BF16, kind="ExternalInput").ap()
    out = nc.dram_tensor("out", [128, 512], F32, kind="ExternalOutput").ap()

    with tile.TileContext(nc) as tc:
        with (
            tc.tile_pool(name="const", bufs=1) as cpool,
            tc.tile_pool(name="xin", bufs=4) as xpool,
            tc.tile_pool(name="gates", bufs=2) as gpool,
            tc.tile_pool(name="state", bufs=2) as spool,
            tc.tile_pool(name="accp", bufs=1) as apool,
            tc.tile_pool(name="parz", bufs=2, space="PSUM") as parz_pool,
            tc.tile_pool(name="pan", bufs=2, space="PSUM") as pan_pool,
            tc.tile_pool(name="pb", bufs=1, space="PSUM") as pb_pool,
            tc.tile_pool(name="pt", bufs=1, space="PSUM") as pt_pool,
        ):
            wx_sb, wh_sb = [], []
            for c in range(4):
                t_ = cpool.tile([128, G3], BF16, tag=f"wx{c}", name=f"wxs{c}")
                nc.sync.dma_start(t_[:], wx[c * 128:(c + 1) * 128, :])
                wx_sb.append(t_)
            for c in range(4):
                t_ = cpool.tile([128, G3], BF16, tag=f"wh{c}", name=f"whs{c}")
                nc.sync.dma_start(t_[:], wh[c * 128:(c + 1) * 128, :])
                wh_sb.append(t_)
            bias_sb = cpool.tile([1, 2048], BF16, tag="biasrow")
            nc.sync.dma_start(bias_sb[:], biasrow[:])
            ones_sb = cpool.tile([1, 128], BF16, tag="ones")
            nc.sync.dma_start(ones_sb[:], onesr[:])
            id_sb = cpool.tile([128, 128], BF16, tag="ident")
            nc.sync.dma_start(id_sb[:], ident[:])
            msk_sb = cpool.tile([128, T], F32, tag="msk")
            nc.sync.dma_start(msk_sb[:], msk[:])

            acc = apool.tile([128, 512], F32, tag="acc")
            nc.vector.memset(acc[:], 0.0)

            hT_prev = spool.tile([128, 512], BF16, tag="hT")
            nc.sync.dma_start(hT_prev[:], hT0[:])
            hg_prev = spool.tile([128, 512], BF16, tag="hg")
            nc.sync.dma_start(hg_prev[:], h0g[:])

            x_tiles = {}

            def fetch_x(t):
                x_t = xpool.tile([128, 512], BF16, tag="x")
                nc.sync.dma_start(x_t[:], xT[:, t * 512:(t + 1) * 512])
                x_tiles[t] = x_t

            parz_tiles = {}
            pan_tiles = {}

            def emit_bias_a(t):
                """Seed step t's gate psum banks with the bias rows."""
                prz = parz_tiles[t]
                for nb in range(2):
                    cols = slice(nb * 512, (nb + 1) * 512)
                    nc.tensor.matmul(prz[:, cols], ones_sb[:],
                                     bias_sb[:, cols], start=True, stop=False)
                nc.tensor.matmul(pan_tiles[t][:], ones_sb[:],
                                 bias_sb[:, 1024:1536], start=True, stop=False)

            def emit_bias_b(p_b):
                nc.tensor.matmul(p_b[:], ones_sb[:], bias_sb[:, 1536:2048],
                                 start=True, stop=False)

            def emit_x(t, region):
                """x-projection of step t for one gate region (0=n,1=r,2=z)."""
                x_t = x_tiles[t]
                nb = (2, 0, 1)[region]
                if nb == 2:
                    tgt, cols = pan_tiles[t], slice(0, 512)
                else:
                    tgt, cols = parz_tiles[t], slice(nb * 512, (nb + 1) * 512)
                for c in range(4):
                    gcols = slice(c * 128, (c + 1) * 128)
                    nc.tensor.matmul(tgt[:, cols], x_t[:, gcols],
                                     wx_sb[c][:, nb * 512:(nb + 1) * 512],
                                     start=False, stop=(nb == 2 and c == 3))

            # preamble: x(0) fully projected before the scan starts
            fetch_x(0)
            fetch_x(1)
            parz_tiles[0] = parz_pool.tile([128, 1024], F32, tag="parz",
                                           name="parz0")
            pan_tiles[0] = pan_pool.tile([128, 512], F32, tag="pan",
                                         name="pan0")
            pb_tiles = {0: pb_pool.tile([128, 512], F32, tag="pb", name="pb0")}
            emit_bias_a(0)
            emit_bias_b(pb_tiles[0])
            for reg in range(3):
                emit_x(0, reg)

            for t in range(T):
                prz = parz_tiles[t]
                pan = pan_tiles[t]
                p_b = pb_tiles[t]
                if t + 2 < T:
                    fetch_x(t + 2)

                # --- recurrent matmuls (wait on hT_prev): r, nh, z ---
                for c in range(4):
                    gcols = slice(c * 128, (c + 1) * 128)
                    nc.tensor.matmul(prz[:, 0:512],
                                     hT_prev[:, gcols],
                                     wh_sb[c][:, 0:512],
                                     start=False, stop=(c == 3))
                for c in range(4):
                    gcols = slice(c * 128, (c + 1) * 128)
                    nc.tensor.matmul(p_b[:], hT_prev[:, gcols],
                                     wh_sb[c][:, 1024:1536],
                                     start=False, stop=(c == 3))
                for c in range(4):
                    gcols = slice(c * 128, (c + 1) * 128)
                    nc.tensor.matmul(prz[:, 512:1024],
                                     hT_prev[:, gcols],
                                     wh_sb[c][:, 512:1024],
                                     start=False, stop=(c == 3))

                # --- ACT: sig-r, sig-z, tanh (tanh emitted below) ---
                r_sb = gpool.tile([128, 512], BF16, tag="r")
                nc.scalar.activation(r_sb[:], prz[:, 0:512], AF.Sigmoid)
                z_sb = gpool.tile([128, 512], BF16, tag="z")
                nc.scalar.activation(z_sb[:], prz[:, 512:1024], AF.Sigmoid)

                # drain p_b to SBUF immediately so its bank is free for the
                # next step's bias (keeps the PE queue from stalling)
                nh_sb = gpool.tile([128, 512], BF16, tag="nh")
                nc.vector.tensor_copy(nh_sb[:], p_b[:])

                # --- x-projection of t+1 fills the PE during the chain ---
                if t + 1 < T:
                    parz_tiles[t + 1] = parz_pool.tile(
                        [128, 1024], F32, tag="parz", name=f"parz{t + 1}")
                    pan_tiles[t + 1] = pan_pool.tile(
                        [128, 512], F32, tag="pan", name=f"pan{t + 1}")
                    pb_tiles[t + 1] = pb_pool.tile([128, 512], F32, tag="pb",
                                                   name=f"pb{t + 1}")
                    emit_bias_a(t + 1)
                    for reg in range(3):
                        emit_x(t + 1, reg)

                t2 = gpool.tile([128, 512], BF16, tag="t2")
                nc.vector.tensor_mul(t2[:], r_sb[:], nh_sb[:])
                t3 = gpool.tile([128, 512], BF16, tag="t3")
                nc.vector.tensor_add(t3[:], t2[:], pan[:])
                n_sb = gpool.tile([128, 512], BF16, tag="n")
                nc.scalar.activation(n_sb[:], t3[:], AF.Tanh)
                # h' = z*h + (1-z)*n   (z', q1 computed off the critical path)
                zc = gpool.tile([128, 512], BF16, tag="zc")
                nc.vector.tensor_scalar(zc[:], z_sb[:], -1.0, 1.0,
                                        op0=ALU.mult, op1=ALU.add)
                q1 = gpool.tile([128, 512], BF16, tag="q1")
                nc.vector.tensor_mul(q1[:], z_sb[:], hg_prev[:])
                q2 = gpool.tile([128, 512], BF16, tag="q2")
                nc.vector.tensor_mul(q2[:], zc[:], n_sb[:])
                hg = spool.tile([128, 512], BF16, tag="hg")
                nc.vector.tensor_add(hg[:], q1[:], q2[:])

                if t + 1 < T:
                    # p_b bias for t+1 here: t2(t) has read p_b by now, and
                    # this keeps the PE busy right before the transposes
                    emit_bias_b(pb_tiles[t + 1])
                    p_t = pt_pool.tile([128, 512], BF16, tag="pt")
                    for c in range(4):
                        gcols = slice(c * 128, (c + 1) * 128)
                        nc.tensor.transpose(p_t[:, gcols],
                                            hg[:, gcols], id_sb[:])
                    hT = spool.tile([128, 512], BF16, tag="hT")
                    nc.vector.tensor_copy(hT[:], p_t[:])
                    hT_prev = hT

                # acc += h' * mask[:, t]
                nc.vector.scalar_tensor_tensor(
                    acc[:], hg[:], msk_sb[:, t:t + 1], acc[:],
                    op0=ALU.mult, op1=ALU.add)
                hg_prev = hg

            nc.sync.dma_start(out[:], acc[:])

    nc.compile()
    return nc


def _host_prep(msg, lengths, block, direction, starts, h0_all, bf):
    """Build one core's input map."""
    gs = block * BPC
    lens = lengths[gs:gs + BPC]
    sts = starts[gs:gs + BPC]

    xpad = np.zeros((T, BPC, H), np.float32)
    mask = np.zeros((BPC, T), np.float32)
    node_rows = np.concatenate(
        [np.arange(sts[j], sts[j] + lens[j]) for j in range(BPC)])
    g_idx = np.repeat(np.arange(BPC), lens)
    pos = np.concatenate([np.arange(lens[j]) for j in range(BPC)])
    t_idx = pos if direction == 0 else (T - 1 - pos)
    xpad[t_idx, g_idx] = msg[node_rows]
    if direction == 0:
        mask[g_idx, pos] = 1.0
    else:
        mask[g_idx, T - 1 - pos] = 1.0

    # xT [128, T*512]: row p, col t*512 + c*128 + g  = xpad[t, g, 128c+p]
    xT = np.ascontiguousarray(
        xpad.reshape(T, BPC, 4, 128).transpose(3, 0, 2, 1).reshape(128, T * 512)
    ).astype(bf)

    h0 = h0_all[gs:gs + BPC]                                   # [g, H]
    hT0 = np.ascontiguousarray(
        h0.reshape(BPC, 4, 128).transpose(2, 1, 0).reshape(128, 512)
    ).astype(bf)
    h0g = np.ascontiguousarray(h0).astype(bf)

    return {
        "xT": xT,
        "hT0": hT0,
        "h0g": h0g,
        "msk": mask,
    }


def kernel(**inputs):
    global LAST_RESULTS
    import ml_dtypes
    bf = ml_dtypes.bfloat16

    h = np.asarray(inputs["h"], np.float32)
    lengths = np.asarray(inputs["lengths"]).astype(np.int64)
    bias = np.asarray(inputs["bias"], np.float32)

    starts = np.concatenate([[0], np.cumsum(lengths)[:-1]]).astype(np.int64)
    h0_all = np.maximum.reduceat(h, starts, axis=0)            # segment max
    msg = np.maximum(h + bias, 0.0)

    if "nc" not in _CACHE:
        _CACHE["nc"] = _build_program()
    nc = _CACHE["nc"]

    wkeys = {0: ("w_ih_f", "w_hh_f", "b_ih_f", "b_hh_f"),
             1: ("w_ih_b", "w_hh_b", "b_ih_b", "b_hh_b")}
    shared = {}
    for direction in (0, 1):
        kw, kh, kbi, kbh = wkeys[direction]
        w_ih = np.asarray(inputs[kw], np.float32)
        w_hh = np.asarray(inputs[kh], np.float32)
        b_ih = np.asarray(inputs[kbi], np.float32)
        b_hh = np.asarray(inputs[kbh], np.float32)
        brow = np.concatenate(
            [b_ih + np.concatenate([b_hh[:1024], np.zeros(512, np.float32)]),
             b_hh[1024:]]).astype(np.float32)              # [2048]
        shared[direction] = {
            "wx": np.ascontiguousarray(w_ih.T).astype(bf),
            "wh": np.ascontiguousarray(w_hh.T).astype(bf),
            "biasrow": brow.reshape(1, 2048).astype(bf),
        }
    ones = np.ones((1, 128), np.float32).astype(bf)
    ident = np.eye(128, dtype=np.float32).astype(bf)

    in_maps = []
    for core in range(NCORES):
        direction, block = divmod(core, 4)
        im = _host_prep(msg, lengths, block, direction, starts, h0_all, bf)
        im.update(shared[direction])
        im["onesr"] = ones
        im["ident"] = ident
        in_maps.append(im)

    res = bass_utils.run_bass_kernel_spmd(nc, in_maps,
                                          core_ids=list(range(NCORES)))
    LAST_RESULTS = res

    out = np.zeros((B, 2 * H), np.float32)
    for core in range(NCORES):
        direction, block = divmod(core, 4)
        gs = block * BPC
        acc = np.asarray(res.results[core]["out"], np.float32)  # [g, H]
        cols = slice(0, H) if direction == 0 else slice(H, 2 * H)
        out[gs:gs + BPC, cols] = acc
    out /= lengths[:, None].astype(np.float32)
    return out


# revision 27
# speedup vs baseline: 1.2830x; 1.1495x over previous
"""Bidirectional batch-GRU over ragged graph sequences on 8 Trainium2 cores.

Sharding: core = dir*4 + block. Cores 0-3 run the forward GRU on graph
blocks of 128; cores 4-7 run the backward GRU on the same blocks with
time-reversed inputs. All raggedness is carried by host-prepared data
(padded transposed inputs pre-activated with relu(h+bias), per-step
validity masks, segment-max initial state), so one SPMD program serves
all cores.

All matmul operands are bf16 (1 cyc/row at any moving width; rel-err
budget validated at ~6e-3 vs the 2e-2 gate). Per step, the PE runs
bias+x-projection of step t+1 inside step t's recurrence window so the
tensor engine never idles and holds its full p-state clock:
  p_a[g, 0:1536] = bias_row + x_t @ W_ih^T (+ h @ W_hh^T for r,z cols)
  p_b[g, 512]    = b_hn_row + h @ W_hn^T
  r,z = sigmoid(p_a[:, :1024]); n = tanh(gxn + r * p_b)
  h' = n + z*(h - n);  acc += h' * mask[:, t];  hT = transpose(h')
The n-gate x-projection is copied out of PSUM early (ACT) so the DVE
chain runs in bf16 fast mode.
"""

import os
import numpy as np

os.environ.setdefault("NEURON_RT_RESET_CORES", "1")

import concourse.bacc as bacc
import concourse.mybir as mybir
import concourse.tile as tile
from concourse import bass_utils

F32 = mybir.dt.float32
BF16 = mybir.dt.bfloat16
AF = mybir.ActivationFunctionType
ALU = mybir.AluOpType


def _install_ntff_shim():
    """Make trace=True usable: this image's antenv lacks axon_hooks, and
    run_bass_kernel_spmd hard-imports it when tracing is requested."""
    try:
        import antenv.axon_hooks  # noqa: F401
        return
    except ImportError:
        pass
    try:
        import sys
        import types
        import antenv
        mod = types.ModuleType("antenv.axon_hooks")
        mod._hook = None
        mod.set_axon_ntff_profile_hook = lambda h: setattr(mod, "_hook", h)
        mod.get_axon_ntff_profile_hook = lambda: mod._hook
        sys.modules["antenv.axon_hooks"] = mod
        antenv.axon_hooks = mod
        from trn_agent_boot.trn_boot import _ntff_profile_via_ctypes
        hook = _ntff_profile_via_ctypes("/opt/axon/libaxon_pjrt.so")
        if hook is not None:
            mod.set_axon_ntff_profile_hook(hook)
    except Exception:
        pass


_install_ntff_shim()

B, T, H = 512, 128, 512
G3 = 3 * H
BPC = 128             # graphs per core
NCORES = 8
MM_MODE = "bf16"

_CACHE = {}
LAST_RESULTS = None


def _build_program():
    nc = bacc.Bacc("TRN2", target_bir_lowering=False, debug=False,
                   num_devices=NCORES)
    xT = nc.dram_tensor("xT", [128, T * 512], BF16, kind="ExternalInput").ap()
    wx = nc.dram_tensor("wx", [512, G3], BF16, kind="ExternalInput").ap()
    wh = nc.dram_tensor("wh", [512, G3], BF16, kind="ExternalInput").ap()
    biasrow = nc.dram_tensor("biasrow", [1, 2048], BF16,
                             kind="ExternalInput").ap()
    onesr = nc.dram_tensor("onesr", [1, 128], BF16, kind="ExternalInput").ap()
    hT0 = nc.dram_tensor("hT0", [128, 512], BF16, kind="ExternalInput").ap()
    h0g = nc.dram_tensor("h0g", [128, 512], BF16, kind="ExternalInput").ap()
    msk = nc.dram_tensor("msk", [128, T], F32, kind="ExternalInput").ap()
    ident = nc.dram_tensor("ident", [128, 128], BF16, kind="ExternalInput").ap()
    out = nc.dram_tensor("out", [128, 512], F32, kind="ExternalOutput").ap()

    with tile.TileContext(nc) as tc:
        with (
            tc.tile_pool(name="const", bufs=1) as cpool,
            tc.tile_pool(name="xin", bufs=4) as xpool,
            tc.tile_pool(name="gates", bufs=2) as gpool,
            tc.tile_pool(name="state", bufs=2) as spool,
            tc.tile_pool(name="accp", bufs=1) as apool,
            tc.tile_pool(name="parz", bufs=2, space="PSUM") as parz_pool,
            tc.tile_pool(name="pan", bufs=2, space="PSUM") as pan_pool,
            tc.tile_pool(name="pb", bufs=1, space="PSUM") as pb_pool,
            tc.tile_pool(name="pt", bufs=1, space="PSUM") as pt_pool,
        ):
            wx_sb, wh_sb = [], []
            for c in range(4):
                t_ = cpool.tile([128, G3], BF16, tag=f"wx{c}", name=f"wxs{c}")
                nc.sync.dma_start(t_[:], wx[c * 128:(c + 1) * 128, :])
                wx_sb.append(t_)
            for c in range(4):
                t_ = cpool.tile([128, G3], BF16, tag=f"wh{c}", name=f"whs{c}")
                nc.sync.dma_start(t_[:], wh[c * 128:(c + 1) * 128, :])
                wh_sb.append(t_)
            bias_sb = cpool.tile([1, 2048], BF16, tag="biasrow")
            nc.sync.dma_start(bias_sb[:], biasrow[:])
            ones_sb = cpool.tile([1, 128], BF16, tag="ones")
            nc.sync.dma_start(ones_sb[:], onesr[:])
            id_sb = cpool.tile([128, 128], BF16, tag="ident")
            nc.sync.dma_start(id_sb[:], ident[:])
            msk_sb = cpool.tile([128, T], F32, tag="msk")
            nc.sync.dma_start(msk_sb[:], msk[:])

            acc = apool.tile([128, 512], F32, tag="acc")
            nc.vector.memset(acc[:], 0.0)

            hT_prev = spool.tile([128, 512], BF16, tag="hT")
            nc.sync.dma_start(hT_prev[:], hT0[:])
            hg_prev = spool.tile([128, 512], BF16, tag="hg")
            nc.sync.dma_start(hg_prev[:], h0g[:])

            x_tiles = {}

            def fetch_x(t):
                x_t = xpool.tile([128, 512], BF16, tag="x")
                nc.sync.dma_start(x_t[:], xT[:, t * 512:(t + 1) * 512])
                x_tiles[t] = x_t

            parz_tiles = {}
            pan_tiles = {}

            def emit_bias_a(t):
                """Seed step t's gate psum banks with the bias rows."""
                prz = parz_tiles[t]
                for nb in range(2):
                    cols = slice(nb * 512, (nb + 1) * 512)
                    nc.tensor.matmul(prz[:, cols], ones_sb[:],
                                     bias_sb[:, cols], start=True, stop=False)
                nc.tensor.matmul(pan_tiles[t][:], ones_sb[:],
                                 bias_sb[:, 1024:1536], start=True, stop=False)

            def emit_bias_b(p_b):
                nc.tensor.matmul(p_b[:], ones_sb[:], bias_sb[:, 1536:2048],
                                 start=True, stop=False)

            def emit_x(t, region):
                """x-projection of step t for one gate region (0=n,1=r,2=z)."""
                x_t = x_tiles[t]
                nb = (2, 0, 1)[region]
                if nb == 2:
                    tgt, cols = pan_tiles[t], slice(0, 512)
                else:
                    tgt, cols = parz_tiles[t], slice(nb * 512, (nb + 1) * 512)
                for c in range(4):
                    gcols = slice(c * 128, (c + 1) * 128)
                    nc.tensor.matmul(tgt[:, cols], x_t[:, gcols],
                                     wx_sb[c][:, nb * 512:(nb + 1) * 512],
                                     start=False, stop=(nb == 2 and c == 3))

            # preamble: x(0) fully projected before the scan starts
            fetch_x(0)
            fetch_x(1)
            parz_tiles[0] = parz_pool.tile([128, 1024], F32, tag="parz",
                                           name="parz0")
            pan_tiles[0] = pan_pool.tile([128, 512], F32, tag="pan",
                                         name="pan0")
            pb_tiles = {0: pb_pool.tile([128, 512], F32, tag="pb", name="pb0")}
            emit_bias_a(0)
            emit_bias_b(pb_tiles[0])
            for reg in range(3):
                emit_x(0, reg)

            for t in range(T):
                prz = parz_tiles[t]
                pan = pan_tiles[t]
                p_b = pb_tiles[t]
                if t + 2 < T:
                    fetch_x(t + 2)

                # --- recurrent matmuls (wait on hT_prev): r, nh, z ---
                for c in range(4):
                    gcols = slice(c * 128, (c + 1) * 128)
                    nc.tensor.matmul(prz[:, 0:512],
                                     hT_prev[:, gcols],
                                     wh_sb[c][:, 0:512],
                                     start=False, stop=(c == 3))
                for c in range(4):
                    gcols = slice(c * 128, (c + 1) * 128)
                    nc.tensor.matmul(p_b[:], hT_prev[:, gcols],
                                     wh_sb[c][:, 1024:1536],
                                     start=False, stop=(c == 3))
                for c in range(4):
                    gcols = slice(c * 128, (c + 1) * 128)
                    nc.tensor.matmul(prz[:, 512:1024],
                                     hT_prev[:, gcols],
                                     wh_sb[c][:, 512:1024],
                                     start=False, stop=(c == 3))

                # --- ACT in column halves so DVE pipelines behind it ---
                r_sb = gpool.tile([128, 512], BF16, tag="r")
                z_sb = gpool.tile([128, 512], BF16, tag="z")
                HA, HB = slice(0, 256), slice(256, 512)
                nc.scalar.activation(r_sb[:, HA], prz[:, 0:256], AF.Sigmoid)
                nc.scalar.activation(r_sb[:, HB], prz[:, 256:512], AF.Sigmoid)
                nc.scalar.activation(z_sb[:, HA], prz[:, 512:768], AF.Sigmoid)

                # drain p_b to SBUF immediately so its bank is free for the
                # next step's bias (keeps the PE queue from stalling)
                nh_sb = gpool.tile([128, 512], BF16, tag="nh")
                nc.vector.tensor_copy(nh_sb[:], p_b[:])

                # --- x-projection of t+1 fills the PE during the chain ---
                if t + 1 < T:
                    parz_tiles[t + 1] = parz_pool.tile(
                        [128, 1024], F32, tag="parz", name=f"parz{t + 1}")
                    pan_tiles[t + 1] = pan_pool.tile(
                        [128, 512], F32, tag="pan", name=f"pan{t + 1}")
                    pb_tiles[t + 1] = pb_pool.tile([128, 512], F32, tag="pb",
                                                   name=f"pb{t + 1}")
                    emit_bias_a(t + 1)
                    for reg in range(3):
                        emit_x(t + 1, reg)

                t2 = gpool.tile([128, 512], BF16, tag="t2")
                t3 = gpool.tile([128, 512], BF16, tag="t3")
                n_sb = gpool.tile([128, 512], BF16, tag="n")
                zc = gpool.tile([128, 512], BF16, tag="zc")
                q1 = gpool.tile([128, 512], BF16, tag="q1")
                q2 = gpool.tile([128, 512], BF16, tag="q2")
                hg = spool.tile([128, 512], BF16, tag="hg")
                p_t = None
                hT = None
                if t + 1 < T:
                    emit_bias_b(pb_tiles[t + 1])
                    p_t = pt_pool.tile([128, 512], BF16, tag="pt")
                    hT = spool.tile([128, 512], BF16, tag="hT")
                # half A down the n-gate chain
                nc.vector.tensor_mul(t2[:, HA], r_sb[:, HA], nh_sb[:, HA])
                nc.vector.tensor_add(t3[:, HA], t2[:, HA], pan[:, 0:256])
                nc.scalar.activation(n_sb[:, HA], t3[:, HA], AF.Tanh)
                # half B + remaining z sigmoid interleave on ACT
                nc.vector.tensor_mul(t2[:, HB], r_sb[:, HB], nh_sb[:, HB])
                nc.vector.tensor_add(t3[:, HB], t2[:, HB], pan[:, 256:512])
                nc.scalar.activation(z_sb[:, HB], prz[:, 768:1024], AF.Sigmoid)
                nc.scalar.activation(n_sb[:, HB], t3[:, HB], AF.Tanh)
                # h' = z*h + (1-z)*n per half; transpose/copy per 128-chunk
                for half, (hs, ps) in enumerate(
                        ((HA, slice(0, 256)), (HB, slice(256, 512)))):
                    nc.vector.tensor_scalar(zc[:, hs], z_sb[:, hs], -1.0, 1.0,
                                            op0=ALU.mult, op1=ALU.add)
                    nc.vector.tensor_mul(q1[:, hs], z_sb[:, hs],
                                         hg_prev[:, hs])
                    nc.vector.tensor_mul(q2[:, hs], zc[:, hs], n_sb[:, hs])
                    nc.vector.tensor_add(hg[:, hs], q1[:, hs], q2[:, hs])
                    if t + 1 < T:
                        for c in (2 * half, 2 * half + 1):
                            gcols = slice(c * 128, (c + 1) * 128)
                            nc.tensor.transpose(p_t[:, gcols],
                                                hg[:, gcols], id_sb[:])
                            nc.vector.tensor_copy(hT[:, gcols], p_t[:, gcols])
                if t + 1 < T:
                    hT_prev = hT

                # acc += h' * mask[:, t]
                nc.vector.scalar_tensor_tensor(
                    acc[:], hg[:], msk_sb[:, t:t + 1], acc[:],
                    op0=ALU.mult, op1=ALU.add)
                hg_prev = hg

            nc.sync.dma_start(out[:], acc[:])

    nc.compile()
    return nc


def _host_prep(msg, lengths, block, direction, starts, h0_all, bf):
    """Build one core's input map."""
    gs = block * BPC
    lens = lengths[gs:gs + BPC]
    sts = starts[gs:gs + BPC]

    xpad = np.zeros((T, BPC, H), np.float32)
    mask = np.zeros((BPC, T), np.float32)
    node_rows = np.concatenate(
        [np.arange(sts[j], sts[j] + lens[j]) for j in range(BPC)])
    g_idx = np.repeat(np.arange(BPC), lens)
    pos = np.concatenate([np.arange(lens[j]) for j in range(BPC)])
    t_idx = pos if direction == 0 else (T - 1 - pos)
    xpad[t_idx, g_idx] = msg[node_rows]
    if direction == 0:
        mask[g_idx, pos] = 1.0
    else:
        mask[g_idx, T - 1 - pos] = 1.0

    # xT [128, T*512]: row p, col t*512 + c*128 + g  = xpad[t, g, 128c+p]
    xT = np.ascontiguousarray(
        xpad.reshape(T, BPC, 4, 128).transpose(3, 0, 2, 1).reshape(128, T * 512)
    ).astype(bf)

    h0 = h0_all[gs:gs + BPC]                                   # [g, H]
    hT0 = np.ascontiguousarray(
        h0.reshape(BPC, 4, 128).transpose(2, 1, 0).reshape(128, 512)
    ).astype(bf)
    h0g = np.ascontiguousarray(h0).astype(bf)

    return {
        "xT": xT,
        "hT0": hT0,
        "h0g": h0g,
        "msk": mask,
    }


def kernel(**inputs):
    global LAST_RESULTS
    import ml_dtypes
    bf = ml_dtypes.bfloat16

    h = np.asarray(inputs["h"], np.float32)
    lengths = np.asarray(inputs["lengths"]).astype(np.int64)
    bias = np.asarray(inputs["bias"], np.float32)

    starts = np.concatenate([[0], np.cumsum(lengths)[:-1]]).astype(np.int64)
    h0_all = np.maximum.reduceat(h, starts, axis=0)            # segment max
    msg = np.maximum(h + bias, 0.0)

    if "nc" not in _CACHE:
        _CACHE["nc"] = _build_program()
    nc = _CACHE["nc"]

    wkeys = {0: ("w_ih_f", "w_hh_f", "b_ih_f", "b_hh_f"),
             1: ("w_ih_b", "w_hh_b", "b_ih_b", "b_hh_b")}
    shared = {}
    for direction in (0, 1):
        kw, kh, kbi, kbh = wkeys[direction]
        w_ih = np.asarray(inputs[kw], np.float32)
        w_hh = np.asarray(inputs[kh], np.float32)
        b_ih = np.asarray(inputs[kbi], np.float32)
        b_hh = np.asarray(inputs[kbh], np.float32)
        brow = np.concatenate(
            [b_ih + np.concatenate([b_hh[:1024], np.zeros(512, np.float32)]),
             b_hh[1024:]]).astype(np.float32)              # [2048]
        shared[direction] = {
            "wx": np.ascontiguousarray(w_ih.T).astype(bf),
            "wh": np.ascontiguousarray(w_hh.T).astype(bf),
            "biasrow": brow.reshape(1, 2048).astype(bf),
        }
    ones = np.ones((1, 128), np.float32).astype(bf)
    ident = np.eye(128, dtype=np.float32).astype(bf)

    in_maps = []
    for core in range(NCORES):
        direction, block = divmod(core, 4)
        im = _host_prep(msg, lengths, block, direction, starts, h0_all, bf)
        im.update(shared[direction])
        im["onesr"] = ones
        im["ident"] = ident
        in_maps.append(im)

    res = bass_utils.run_bass_kernel_spmd(nc, in_maps,
                                          core_ids=list(range(NCORES)))
    LAST_RESULTS = res

    out = np.zeros((B, 2 * H), np.float32)
    for core in range(NCORES):
        direction, block = divmod(core, 4)
        gs = block * BPC
        acc = np.asarray(res.results[core]["out"], np.float32)  # [g, H]
        cols = slice(0, H) if direction == 0 else slice(H, 2 * H)
        out[gs:gs + BPC, cols] = acc
    out /= lengths[:, None].astype(np.float32)
    return out
